# revision 3
# baseline (speedup 1.0000x reference)
"""Trainium2 Bass kernel for nn_AttentionUnit_v2 (sparse_attention).

Per batch b:
    tq    = W @ q_b + bias                    (fp32 on PE)
    alpha = keys_b @ tq                       (fp16 inputs, fp32 PSUM accum)
    alpha = where(mask, -80, alpha); softmax  (fp32 DVE/ACT)
    att   = alpha_sm @ keys_b                 (fp16 inputs, fp32 PSUM accum)

Sharding: pure data parallel, 2048 batches -> 8 cores x 256.
Keys are sent in BOTH layouts as fp16 (natural [b,n,k] for mm2, transposed
[b,k,n] for mm1) because the PE contracts the partition dim of both operands:
mm1 contracts k, mm2 contracts n.  DMA ~134MB/core => ~375us roofline.

Matvec mapping on the 128x128 PE (per batch):
  mm1: 8 matmuls lhsT=keysT[k-chunk, n-chunk] (128x128), rhs=tq col [128,1]
       -> alphaT PSUM bank [128n, 4, 128b], ONE accum group per 128-batch group
  mm2: 8 matmuls lhsT=keysn[n-chunk, k-half], rhs=alpha_smT col [128,1]
       -> attT PSUM bank [128k, 2, 128b]
Softmax runs on [128 batches, 512] tiles after a PE transpose of alphaT.
"""

import os
import sys
import types

import numpy as np

for _p in ("/opt/trn_rl_repo",):
    if _p not in sys.path:
        sys.path.insert(0, _p)

import concourse.bass as bass  # noqa: E402
import concourse.bacc as bacc  # noqa: E402
import concourse.tile as tile  # noqa: E402
from concourse import mybir  # noqa: E402
from concourse import bass_utils  # noqa: E402

F16 = mybir.dt.float16
F32 = mybir.dt.float32

N_BATCH, QUERY_DIM, KEY_DIM, N_KEYS = 2048, 512, 256, 512
N_CORES = 8
MASK_NEG = np.float32(-80.0)

LAST_EXEC_NS = None
LAST_RESULTS = None


def build(b_core=N_BATCH // N_CORES, group=128, db=8):
    """Build the per-core Bass program. b_core batches, softmax groups of
    `group`, `db` batches per keys DMA."""
    assert b_core % group == 0 and group % db == 0 and group <= 512
    nc = bacc.Bacc(None, target_bir_lowering=False)

    keysn = nc.dram_tensor("keysn", [b_core, N_KEYS, KEY_DIM], F16, kind="ExternalInput")
    keyst = nc.dram_tensor("keyst", [b_core, KEY_DIM, N_KEYS], F16, kind="ExternalInput")
    qT = nc.dram_tensor("qT", [QUERY_DIM, b_core], F32, kind="ExternalInput")
    wT = nc.dram_tensor("wT", [QUERY_DIM, KEY_DIM], F32, kind="ExternalInput")
    biasv = nc.dram_tensor("biasv", [KEY_DIM], F32, kind="ExternalInput")
    maskT = nc.dram_tensor("maskT", [N_KEYS, b_core], F32, kind="ExternalInput")
    identf = nc.dram_tensor("identf", [128, 128], F32, kind="ExternalInput")
    identh = nc.dram_tensor("identh", [128, 128], F16, kind="ExternalInput")
    attened = nc.dram_tensor("attened", [b_core, KEY_DIM], F32, kind="ExternalOutput")
    alpha_sm = nc.dram_tensor("alpha_sm", [b_core, N_KEYS], F32, kind="ExternalOutput")

    NC4 = N_KEYS // 128   # 4 n-chunks
    KC2 = KEY_DIM // 128  # 2 k-chunks
    DC4 = QUERY_DIM // 128  # 4 d-chunks

    with tile.TileContext(nc) as tc:
        with (
            tc.tile_pool(name="singles", bufs=1) as singles,
            tc.tile_pool(name="ktp", bufs=4) as ktp,
            tc.tile_pool(name="knp", bufs=4) as knp,
            tc.tile_pool(name="work", bufs=2) as work,
            tc.tile_pool(name="ps_tq", bufs=1, space="PSUM") as ps_tq,
            tc.tile_pool(name="ps_at", bufs=2, space="PSUM") as ps_at,
            tc.tile_pool(name="ps_al", bufs=1, space="PSUM") as ps_al,
            tc.tile_pool(name="ps_st", bufs=1, space="PSUM") as ps_st,
            tc.tile_pool(name="ps_kt", bufs=2, space="PSUM") as ps_kt,
            tc.tile_pool(name="ps_ka", bufs=1, space="PSUM") as ps_ka,
        ):
            qT_s = singles.tile([128, DC4, b_core], F32)
            nc.sync.dma_start(qT_s, qT.rearrange("(c p) b -> p c b", p=128))
            wT_s = singles.tile([128, DC4, KEY_DIM], F32)
            nc.sync.dma_start(wT_s, wT.rearrange("(c p) k -> p c k", p=128))
            bias_s = singles.tile([128, KC2], F32)
            nc.sync.dma_start(bias_s, biasv.rearrange("(c p) -> p c", p=128))
            maskT_s = singles.tile([128, NC4, b_core], F32)
            nc.sync.dma_start(maskT_s, maskT.rearrange("(c p) b -> p c b", p=128))
            idf_s = singles.tile([128, 128], F32)
            nc.sync.dma_start(idf_s, identf[:, :])
            idh_s = singles.tile([128, 128], F16)
            nc.sync.dma_start(idh_s, identh[:, :])

            for g in range(b_core // group):
                g0 = g * group

                # ---- linear: tqT[k, b] = W.T chunks.T @ qT chunks (fp32) ----
                tqT_ps = ps_tq.tile([128, KC2, group], F32, tag="tq")
                for kh in range(KC2):
                    for dc in range(DC4):
                        nc.tensor.matmul(
                            tqT_ps[:, kh, :],
                            lhsT=wT_s[:, dc, kh * 128:(kh + 1) * 128],
                            rhs=qT_s[:, dc, g0:g0 + group],
                            start=(kh == 0 and dc == 0),
                            stop=(kh == KC2 - 1 and dc == DC4 - 1),
                        )
                tqT_h = work.tile([128, KC2, group], F16, tag="tqh")
                for kh in range(KC2):
                    nc.vector.tensor_scalar_add(
                        tqT_h[:, kh, :], tqT_ps[:, kh, :], bias_s[:, kh:kh + 1]
                    )

                # ---- mm1: alphaT[n, b] += keysT_chunk.T @ tq col ----
                alphaT_ps = ps_at.tile([128, NC4, group], F32, tag="alphaT")
                for t in range(group // db):
                    kt_t = ktp.tile([128, db, KC2, N_KEYS], F16, tag="kt")
                    nc.sync.dma_start(
                        kt_t,
                        keyst[g0 + t * db:g0 + (t + 1) * db].rearrange(
                            "b (c p) n -> p b c n", p=128
                        ),
                    )
                    for j in range(db):
                        jj = t * db + j
                        for nci in range(NC4):
                            for kc in range(KC2):
                                nc.tensor.matmul(
                                    alphaT_ps[:, nci, jj:jj + 1],
                                    lhsT=kt_t[:, j, kc, nci * 128:(nci + 1) * 128],
                                    rhs=tqT_h[:, kc, jj:jj + 1],
                                    start=(jj == 0 and nci == 0 and kc == 0),
                                    stop=(jj == group - 1 and nci == NC4 - 1
                                          and kc == KC2 - 1),
                                )

                # ---- mask add + evacuate PSUM -> SBUF ----
                alphaTm = work.tile([128, NC4, group], F32, tag="alphaTm")
                for c in range(NC4):
                    nc.vector.tensor_add(
                        alphaTm[:, c, :], alphaT_ps[:, c, :],
                        maskT_s[:, c, g0:g0 + group],
                    )

                # ---- transpose alphaT -> alpha [b, n] (PE, fp32) ----
                alpha_ps = ps_al.tile([group, N_KEYS], F32, tag="alpha")
                for c in range(NC4):
                    nc.tensor.matmul(
                        alpha_ps[:, c * 128:(c + 1) * 128],
                        lhsT=alphaTm[:, c, :], rhs=idf_s[:, :],
                        is_transpose=True,
                        start=(c == 0), stop=(c == NC4 - 1),
                    )

                # ---- softmax over n (rows = batches) ----
                negmax = work.tile([group, 1], F32, tag="negmax")
                nc.vector.reduce_max(
                    negmax, alpha_ps, axis=mybir.AxisListType.X, negate=True
                )
                expt = work.tile([group, N_KEYS], F32, tag="expt")
                sume = work.tile([group, 1], F32, tag="sume")
                nc.scalar.activation(
                    expt, alpha_ps, mybir.ActivationFunctionType.Exp,
                    bias=negmax, accum_out=sume,
                )
                rsum = work.tile([group, 1], F32, tag="rsum")
                nc.vector.reciprocal(rsum, sume)
                asm_f = work.tile([group, N_KEYS], F32, tag="asm_f")
                nc.vector.tensor_scalar_mul(asm_f, expt, rsum)
                asm_h = work.tile([group, N_KEYS], F16, tag="asm_h")
                nc.vector.tensor_scalar_mul(asm_h, expt, rsum)
                nc.sync.dma_start(alpha_sm[g0:g0 + group, :], asm_f)

                # ---- transpose alpha_sm -> asmT [n, b] (PE, fp16) ----
                asmT_ps = ps_st.tile([128, NC4, group], F16, tag="asmT")
                for c in range(NC4):
                    nc.tensor.matmul(
                        asmT_ps[:, c, :],
                        lhsT=asm_h[:, c * 128:(c + 1) * 128],
                        rhs=idh_s[:group, :group],
                        is_transpose=True,
                        start=(c == 0), stop=(c == NC4 - 1),
                    )
                asmT_s = work.tile([128, NC4, group], F16, tag="asmT_s")
                for c in range(NC4):
                    nc.scalar.copy(asmT_s[:, c, :], asmT_ps[:, c, :])

                # ---- mm2: attT[k, b] += keysn_chunk.T @ asmT col ----
                attT_ps = ps_kt.tile([128, KC2, group], F32, tag="attT")
                for t in range(group // db):
                    kn_t = knp.tile([128, db, NC4, KEY_DIM], F16, tag="kn")
                    nc.sync.dma_start(
                        kn_t,
                        keysn[g0 + t * db:g0 + (t + 1) * db].rearrange(
                            "b (c p) k -> p b c k", p=128
                        ),
                    )
                    for j in range(db):
                        jj = t * db + j
                        for kh in range(KC2):
                            for c in range(NC4):
                                nc.tensor.matmul(
                                    attT_ps[:, kh, jj:jj + 1],
                                    lhsT=kn_t[:, j, c, kh * 128:(kh + 1) * 128],
                                    rhs=asmT_s[:, c, jj:jj + 1],
                                    start=(jj == 0 and kh == 0 and c == 0),
                                    stop=(jj == group - 1 and kh == KC2 - 1
                                          and c == NC4 - 1),
                                )

                # ---- attened: evac, transpose to [b, k], store ----
                attT_s = work.tile([128, KC2, group], F32, tag="attT_s")
                for kh in range(KC2):
                    nc.scalar.copy(attT_s[:, kh, :], attT_ps[:, kh, :])
                att_ps = ps_ka.tile([group, KEY_DIM], F32, tag="att")
                for kh in range(KC2):
                    nc.tensor.matmul(
                        att_ps[:, kh * 128:(kh + 1) * 128],
                        lhsT=attT_s[:, kh, :], rhs=idf_s[:, :],
                        is_transpose=True,
                        start=(kh == 0), stop=(kh == KC2 - 1),
                    )
                att_s = work.tile([group, KEY_DIM], F32, tag="att_s")
                nc.scalar.copy(att_s, att_ps)
                nc.sync.dma_start(attened[g0:g0 + group, :], att_s)

    nc.compile()
    return nc


_NC_CACHE = {}


def _get_nc():
    if "nc" not in _NC_CACHE:
        _NC_CACHE["nc"] = build()
    return _NC_CACHE["nc"]


def _install_trace_shims():
    """The agent image lacks antenv.axon_hooks; provide it so trace=True works,
    and stub the S3 artifact upload."""
    if "antenv.axon_hooks" not in sys.modules:
        import trn_agent_boot.trn_boot as tb

        mod = types.ModuleType("antenv.axon_hooks")
        hook = tb._ntff_profile_via_ctypes("/opt/axon/libaxon_pjrt.so")
        mod.get_axon_ntff_profile_hook = lambda: hook
        mod.set_axon_ntff_profile_hook = lambda h: None
        sys.modules["antenv.axon_hooks"] = mod
    bass_utils.upload_artifacts = lambda tmpdir: "local://skipped"


def make_in_maps(queries, keys, null_mask, W, b, n_cores=N_CORES):
    queries = np.asarray(queries, dtype=np.float32)
    keys = np.asarray(keys, dtype=np.float32)
    null_mask = np.asarray(null_mask)
    W = np.asarray(W, dtype=np.float32)
    b = np.asarray(b, dtype=np.float32)

    kn = keys.astype(np.float16)
    kt = np.ascontiguousarray(keys.transpose(0, 2, 1)).astype(np.float16)
    qT = np.ascontiguousarray(queries.T)
    wT = np.ascontiguousarray(W.T)
    maskT = np.ascontiguousarray(null_mask.T).astype(np.float32) * MASK_NEG
    identf = np.eye(128, dtype=np.float32)
    identh = np.eye(128, dtype=np.float16)

    nb = queries.shape[0]
    per = nb // n_cores
    in_maps = []
    for i in range(n_cores):
        sl = slice(i * per, (i + 1) * per)
        in_maps.append({
            "keysn": np.ascontiguousarray(kn[sl]),
            "keyst": np.ascontiguousarray(kt[sl]),
            "qT": np.ascontiguousarray(qT[:, sl]),
            "wT": wT,
            "biasv": b,
            "maskT": np.ascontiguousarray(maskT[:, sl]),
            "identf": identf,
            "identh": identh,
        })
    return in_maps


def kernel(queries, keys, trans_keys=None, null_mask=None, W=None, b=None):
    global LAST_EXEC_NS, LAST_RESULTS
    in_maps = make_in_maps(queries, keys, null_mask, W, b)
    nc = _get_nc()

    trace = os.environ.get("KERNEL_TRACE", "0") == "1"
    if trace:
        _install_trace_shims()
    res = bass_utils.run_bass_kernel_spmd(
        nc, in_maps, core_ids=list(range(N_CORES)), trace=trace
    )
    LAST_EXEC_NS = res.exec_time_ns
    LAST_RESULTS = res
    attened = np.concatenate([r["attened"] for r in res.results], axis=0)
    alpha_sm = np.concatenate([r["alpha_sm"] for r in res.results], axis=0)
    return attened.astype(np.float32), alpha_sm.astype(np.float32)


# revision 7
# speedup vs baseline: 1.2848x; 1.2848x over previous
"""Trainium2 Bass kernel for nn_AttentionUnit_v2 (sparse_attention).

Per batch b:
    tq    = W @ q_b + bias                    (fp32 on PE)
    alpha = keys_b @ tq                       (fp16 inputs, fp32 PSUM accum)
    alpha = where(mask, -80, alpha); softmax  (fp32 DVE/ACT)
    att   = alpha_sm @ keys_b                 (fp16 inputs, fp32 PSUM accum)

Sharding: pure data parallel, 2048 batches -> 8 cores x 256.
Keys are sent in BOTH layouts as fp16 (natural [b,n,k] for mm2, transposed
[b,k,n] for mm1) because the PE contracts the partition dim of both operands:
mm1 contracts k, mm2 contracts n.  DMA ~134MB/core => ~375us roofline.

Matvec mapping on the 128x128 PE (per batch):
  mm1: 8 matmuls lhsT=keysT[k-chunk, n-chunk] (128x128), rhs=tq col [128,1]
       -> alphaT PSUM bank [128n, 4, 128b], ONE accum group per 128-batch group
  mm2: 8 matmuls lhsT=keysn[n-chunk, k-half], rhs=alpha_smT col [128,1]
       -> attT PSUM bank [128k, 2, 128b]
Softmax runs on [128 batches, 512] tiles after a PE transpose of alphaT.
"""

import os
import sys
import types

import numpy as np

for _p in ("/opt/trn_rl_repo",):
    if _p not in sys.path:
        sys.path.insert(0, _p)

import concourse.bass as bass  # noqa: E402
import concourse.bacc as bacc  # noqa: E402
import concourse.tile as tile  # noqa: E402
from concourse import mybir  # noqa: E402
from concourse import bass_utils  # noqa: E402

F16 = mybir.dt.float16
F32 = mybir.dt.float32

N_BATCH, QUERY_DIM, KEY_DIM, N_KEYS = 2048, 512, 256, 512
N_CORES = 8
MASK_NEG = np.float32(-80.0)

LAST_EXEC_NS = None
LAST_RESULTS = None


def build(b_core=N_BATCH // N_CORES, group=128, db=8):
    """Build the per-core Bass program. b_core batches, softmax groups of
    `group`, `db` batches per keys DMA.

    keysn/keyst arrive pre-packed by the host into the exact SBUF tile
    layout, so every keys DMA is a fully-contiguous [128, db*...] block."""
    assert b_core % group == 0 and group % db == 0 and group <= 512
    nt = b_core // db
    nc = bacc.Bacc(None, target_bir_lowering=False)

    keysn = nc.dram_tensor("keysn", [nt, 128, db * (N_KEYS // 128) * KEY_DIM],
                           F16, kind="ExternalInput")
    keyst = nc.dram_tensor("keyst", [nt, 128, db * (KEY_DIM // 128) * N_KEYS],
                           F16, kind="ExternalInput")
    qT = nc.dram_tensor("qT", [QUERY_DIM, b_core], F32, kind="ExternalInput")
    wT = nc.dram_tensor("wT", [QUERY_DIM, KEY_DIM], F32, kind="ExternalInput")
    biasv = nc.dram_tensor("biasv", [KEY_DIM], F32, kind="ExternalInput")
    maskT = nc.dram_tensor("maskT", [N_KEYS, b_core], F32, kind="ExternalInput")
    identf = nc.dram_tensor("identf", [128, 128], F32, kind="ExternalInput")
    identh = nc.dram_tensor("identh", [128, 128], F16, kind="ExternalInput")
    attened = nc.dram_tensor("attened", [b_core, KEY_DIM], F32, kind="ExternalOutput")
    alpha_sm = nc.dram_tensor("alpha_sm", [b_core, N_KEYS], F32, kind="ExternalOutput")

    NC4 = N_KEYS // 128   # 4 n-chunks
    KC2 = KEY_DIM // 128  # 2 k-chunks
    DC4 = QUERY_DIM // 128  # 4 d-chunks

    with tile.TileContext(nc) as tc:
        with (
            tc.tile_pool(name="singles", bufs=1) as singles,
            tc.tile_pool(name="ktp", bufs=4) as ktp,
            tc.tile_pool(name="knp", bufs=4) as knp,
            tc.tile_pool(name="work", bufs=2) as work,
            tc.tile_pool(name="ps_tq", bufs=1, space="PSUM") as ps_tq,
            tc.tile_pool(name="ps_at", bufs=2, space="PSUM") as ps_at,
            tc.tile_pool(name="ps_al", bufs=1, space="PSUM") as ps_al,
            tc.tile_pool(name="ps_st", bufs=1, space="PSUM") as ps_st,
            tc.tile_pool(name="ps_kt", bufs=2, space="PSUM") as ps_kt,
            tc.tile_pool(name="ps_ka", bufs=1, space="PSUM") as ps_ka,
        ):
            qT_s = singles.tile([128, DC4, b_core], F32)
            nc.sync.dma_start(qT_s, qT.rearrange("(c p) b -> p c b", p=128))
            wT_s = singles.tile([128, DC4, KEY_DIM], F32)
            nc.sync.dma_start(wT_s, wT.rearrange("(c p) k -> p c k", p=128))
            bias_s = singles.tile([128, KC2], F32)
            nc.sync.dma_start(bias_s, biasv.rearrange("(c p) -> p c", p=128))
            maskT_s = singles.tile([128, NC4, b_core], F32)
            nc.sync.dma_start(maskT_s, maskT.rearrange("(c p) b -> p c b", p=128))
            idf_s = singles.tile([128, 128], F32)
            nc.sync.dma_start(idf_s, identf[:, :])
            idh_s = singles.tile([128, 128], F16)
            nc.sync.dma_start(idh_s, identh[:, :])

            for g in range(b_core // group):
                g0 = g * group

                # ---- linear: tqT[k, b] = W.T chunks.T @ qT chunks (fp32) ----
                tqT_ps = ps_tq.tile([128, KC2, group], F32, tag="tq")
                for kh in range(KC2):
                    for dc in range(DC4):
                        nc.tensor.matmul(
                            tqT_ps[:, kh, :],
                            lhsT=wT_s[:, dc, kh * 128:(kh + 1) * 128],
                            rhs=qT_s[:, dc, g0:g0 + group],
                            start=(kh == 0 and dc == 0),
                            stop=(kh == KC2 - 1 and dc == DC4 - 1),
                        )
                tqT_h = work.tile([128, KC2, group], F16, tag="tqh")
                for kh in range(KC2):
                    nc.vector.tensor_scalar_add(
                        tqT_h[:, kh, :], tqT_ps[:, kh, :], bias_s[:, kh:kh + 1]
                    )

                # ---- mm1: alphaT[n, b] += keysT_chunk.T @ tq col ----
                alphaT_ps = ps_at.tile([128, NC4, group], F32, tag="alphaT")
                for t in range(group // db):
                    kt_t = ktp.tile([128, db, KC2, N_KEYS], F16, tag="kt")
                    nc.sync.dma_start(
                        kt_t,
                        keyst[g0 // db + t].rearrange(
                            "p (b c n) -> p b c n", b=db, c=KC2
                        ),
                    )
                    for j in range(db):
                        jj = t * db + j
                        for nci in range(NC4):
                            for kc in range(KC2):
                                nc.tensor.matmul(
                                    alphaT_ps[:, nci, jj:jj + 1],
                                    lhsT=kt_t[:, j, kc, nci * 128:(nci + 1) * 128],
                                    rhs=tqT_h[:, kc, jj:jj + 1],
                                    start=(jj == 0 and nci == 0 and kc == 0),
                                    stop=(jj == group - 1 and nci == NC4 - 1
                                          and kc == KC2 - 1),
                                )

                # ---- mask add + evacuate PSUM -> SBUF ----
                alphaTm = work.tile([128, NC4, group], F32, tag="alphaTm")
                for c in range(NC4):
                    nc.vector.tensor_add(
                        alphaTm[:, c, :], alphaT_ps[:, c, :],
                        maskT_s[:, c, g0:g0 + group],
                    )

                # ---- transpose alphaT -> alpha [b, n] (PE, fp32) ----
                alpha_ps = ps_al.tile([group, N_KEYS], F32, tag="alpha")
                for c in range(NC4):
                    nc.tensor.matmul(
                        alpha_ps[:, c * 128:(c + 1) * 128],
                        lhsT=alphaTm[:, c, :], rhs=idf_s[:, :],
                        is_transpose=True,
                        start=(c == 0), stop=(c == NC4 - 1),
                    )

                # ---- softmax over n (rows = batches) ----
                negmax = work.tile([group, 1], F32, tag="negmax")
                nc.vector.reduce_max(
                    negmax, alpha_ps, axis=mybir.AxisListType.X, negate=True
                )
                expt = work.tile([group, N_KEYS], F32, tag="expt")
                sume = work.tile([group, 1], F32, tag="sume")
                nc.scalar.activation(
                    expt, alpha_ps, mybir.ActivationFunctionType.Exp,
                    bias=negmax, accum_out=sume,
                )
                rsum = work.tile([group, 1], F32, tag="rsum")
                nc.vector.reciprocal(rsum, sume)
                asm_f = work.tile([group, N_KEYS], F32, tag="asm_f")
                nc.vector.tensor_scalar_mul(asm_f, expt, rsum)
                asm_h = work.tile([group, N_KEYS], F16, tag="asm_h")
                nc.vector.tensor_scalar_mul(asm_h, expt, rsum)
                nc.sync.dma_start(alpha_sm[g0:g0 + group, :], asm_f)

                # ---- transpose alpha_sm -> asmT [n, b] (PE, fp16) ----
                asmT_ps = ps_st.tile([128, NC4, group], F16, tag="asmT")
                for c in range(NC4):
                    nc.tensor.matmul(
                        asmT_ps[:, c, :],
                        lhsT=asm_h[:, c * 128:(c + 1) * 128],
                        rhs=idh_s[:group, :group],
                        is_transpose=True,
                        start=(c == 0), stop=(c == NC4 - 1),
                    )
                asmT_s = work.tile([128, NC4, group], F16, tag="asmT_s")
                for c in range(NC4):
                    nc.scalar.copy(asmT_s[:, c, :], asmT_ps[:, c, :])

                # ---- mm2: attT[k, b] += keysn_chunk.T @ asmT col ----
                attT_ps = ps_kt.tile([128, KC2, group], F32, tag="attT")
                for t in range(group // db):
                    kn_t = knp.tile([128, db, NC4, KEY_DIM], F16, tag="kn")
                    nc.sync.dma_start(
                        kn_t,
                        keysn[g0 // db + t].rearrange(
                            "p (b c k) -> p b c k", b=db, c=NC4
                        ),
                    )
                    for j in range(db):
                        jj = t * db + j
                        for kh in range(KC2):
                            for c in range(NC4):
                                nc.tensor.matmul(
                                    attT_ps[:, kh, jj:jj + 1],
                                    lhsT=kn_t[:, j, c, kh * 128:(kh + 1) * 128],
                                    rhs=asmT_s[:, c, jj:jj + 1],
                                    start=(jj == 0 and kh == 0 and c == 0),
                                    stop=(jj == group - 1 and kh == KC2 - 1
                                          and c == NC4 - 1),
                                )

                # ---- attened: evac, transpose to [b, k], store ----
                attT_s = work.tile([128, KC2, group], F32, tag="attT_s")
                for kh in range(KC2):
                    nc.scalar.copy(attT_s[:, kh, :], attT_ps[:, kh, :])
                att_ps = ps_ka.tile([group, KEY_DIM], F32, tag="att")
                for kh in range(KC2):
                    nc.tensor.matmul(
                        att_ps[:, kh * 128:(kh + 1) * 128],
                        lhsT=attT_s[:, kh, :], rhs=idf_s[:, :],
                        is_transpose=True,
                        start=(kh == 0), stop=(kh == KC2 - 1),
                    )
                att_s = work.tile([group, KEY_DIM], F32, tag="att_s")
                nc.scalar.copy(att_s, att_ps)
                nc.sync.dma_start(attened[g0:g0 + group, :], att_s)

    nc.compile()
    return nc


_NC_CACHE = {}


def _get_nc():
    if "nc" not in _NC_CACHE:
        _NC_CACHE["nc"] = build()
    return _NC_CACHE["nc"]


def _install_trace_shims():
    """The agent image lacks antenv.axon_hooks; provide it so trace=True works,
    and stub the S3 artifact upload."""
    if "antenv.axon_hooks" not in sys.modules:
        import trn_agent_boot.trn_boot as tb

        mod = types.ModuleType("antenv.axon_hooks")
        hook = tb._ntff_profile_via_ctypes("/opt/axon/libaxon_pjrt.so")
        mod.get_axon_ntff_profile_hook = lambda: hook
        mod.set_axon_ntff_profile_hook = lambda h: None
        sys.modules["antenv.axon_hooks"] = mod
    bass_utils.upload_artifacts = lambda tmpdir: "local://skipped"


def pack_keysn(kn, db=8):
    """[B, 512, 256] f16 -> [B/db, 128, db*4*256] matching the SBUF tile
    layout [p, j, c, k] with n = c*128 + p."""
    B = kn.shape[0]
    v = kn.reshape(B // db, db, N_KEYS // 128, 128, KEY_DIM)
    v = v.transpose(0, 3, 1, 2, 4)
    return np.ascontiguousarray(v).reshape(B // db, 128, -1)


def pack_keyst(kt, db=8):
    """[B, 256, 512] f16 -> [B/db, 128, db*2*512] matching the SBUF tile
    layout [p, j, kc, n] with k = kc*128 + p."""
    B = kt.shape[0]
    v = kt.reshape(B // db, db, KEY_DIM // 128, 128, N_KEYS)
    v = v.transpose(0, 3, 1, 2, 4)
    return np.ascontiguousarray(v).reshape(B // db, 128, -1)


def make_in_maps(queries, keys, null_mask, W, b, n_cores=N_CORES, db=8):
    queries = np.asarray(queries, dtype=np.float32)
    keys = np.asarray(keys, dtype=np.float32)
    null_mask = np.asarray(null_mask)
    W = np.asarray(W, dtype=np.float32)
    b = np.asarray(b, dtype=np.float32)

    kn = keys.astype(np.float16)
    kt = np.ascontiguousarray(keys.transpose(0, 2, 1)).astype(np.float16)
    qT = np.ascontiguousarray(queries.T)
    wT = np.ascontiguousarray(W.T)
    maskT = np.ascontiguousarray(null_mask.T).astype(np.float32) * MASK_NEG
    identf = np.eye(128, dtype=np.float32)
    identh = np.eye(128, dtype=np.float16)

    nb = queries.shape[0]
    per = nb // n_cores
    in_maps = []
    for i in range(n_cores):
        sl = slice(i * per, (i + 1) * per)
        in_maps.append({
            "keysn": pack_keysn(kn[sl], db),
            "keyst": pack_keyst(kt[sl], db),
            "qT": np.ascontiguousarray(qT[:, sl]),
            "wT": wT,
            "biasv": b,
            "maskT": np.ascontiguousarray(maskT[:, sl]),
            "identf": identf,
            "identh": identh,
        })
    return in_maps


def kernel(queries, keys, trans_keys=None, null_mask=None, W=None, b=None):
    global LAST_EXEC_NS, LAST_RESULTS
    in_maps = make_in_maps(queries, keys, null_mask, W, b)
    nc = _get_nc()

    trace = os.environ.get("KERNEL_TRACE", "0") == "1"
    if trace:
        _install_trace_shims()
    res = bass_utils.run_bass_kernel_spmd(
        nc, in_maps, core_ids=list(range(N_CORES)), trace=trace
    )
    LAST_EXEC_NS = res.exec_time_ns
    LAST_RESULTS = res
    attened = np.concatenate([r["attened"] for r in res.results], axis=0)
    alpha_sm = np.concatenate([r["alpha_sm"] for r in res.results], axis=0)
    return attened.astype(np.float32), alpha_sm.astype(np.float32)


# revision 8
# speedup vs baseline: 1.3263x; 1.0323x over previous
"""Trainium2 Bass kernel for nn_AttentionUnit_v2 (sparse_attention).

Per batch b:
    tq    = W @ q_b + bias                    (fp32 on PE)
    alpha = keys_b @ tq                       (fp16 inputs, fp32 PSUM accum)
    alpha = where(mask, -80, alpha); softmax  (fp32 DVE/ACT)
    att   = alpha_sm @ keys_b                 (fp16 inputs, fp32 PSUM accum)

Sharding: pure data parallel, 2048 batches -> 8 cores x 256.
Keys are sent in BOTH layouts as fp16 (natural [b,n,k] for mm2, transposed
[b,k,n] for mm1) because the PE contracts the partition dim of both operands:
mm1 contracts k, mm2 contracts n.  DMA ~134MB/core => ~375us roofline.

Matvec mapping on the 128x128 PE (per batch):
  mm1: 8 matmuls lhsT=keysT[k-chunk, n-chunk] (128x128), rhs=tq col [128,1]
       -> alphaT PSUM bank [128n, 4, 128b], ONE accum group per 128-batch group
  mm2: 8 matmuls lhsT=keysn[n-chunk, k-half], rhs=alpha_smT col [128,1]
       -> attT PSUM bank [128k, 2, 128b]
Softmax runs on [128 batches, 512] tiles after a PE transpose of alphaT.
"""

import os
import sys
import types

import numpy as np

for _p in ("/opt/trn_rl_repo",):
    if _p not in sys.path:
        sys.path.insert(0, _p)

import concourse.bass as bass  # noqa: E402
import concourse.bacc as bacc  # noqa: E402
import concourse.tile as tile  # noqa: E402
from concourse import mybir  # noqa: E402
from concourse import bass_utils  # noqa: E402

F16 = mybir.dt.float16
F32 = mybir.dt.float32

N_BATCH, QUERY_DIM, KEY_DIM, N_KEYS = 2048, 512, 256, 512
N_CORES = 8
MASK_NEG = np.float32(-80.0)

LAST_EXEC_NS = None
LAST_RESULTS = None


def build(b_core=N_BATCH // N_CORES, group=128, db=4):
    """Build the per-core Bass program. b_core batches, softmax groups of
    `group`, `db` batches per keys DMA.

    keysn/keyst arrive pre-packed by the host into the exact SBUF tile
    layout, so every keys DMA is a fully-contiguous [128, db*...] block."""
    assert b_core % group == 0 and group % db == 0 and group <= 512
    nt = b_core // db
    nc = bacc.Bacc(None, target_bir_lowering=False)

    keysn = nc.dram_tensor("keysn", [nt, 128, db * (N_KEYS // 128) * KEY_DIM],
                           F16, kind="ExternalInput")
    keyst = nc.dram_tensor("keyst", [nt, 128, db * (KEY_DIM // 128) * N_KEYS],
                           F16, kind="ExternalInput")
    qT = nc.dram_tensor("qT", [QUERY_DIM, b_core], F32, kind="ExternalInput")
    wT = nc.dram_tensor("wT", [QUERY_DIM, KEY_DIM], F32, kind="ExternalInput")
    biasv = nc.dram_tensor("biasv", [KEY_DIM], F32, kind="ExternalInput")
    maskT = nc.dram_tensor("maskT", [N_KEYS, b_core], F32, kind="ExternalInput")
    identf = nc.dram_tensor("identf", [128, 128], F32, kind="ExternalInput")
    identh = nc.dram_tensor("identh", [128, 128], F16, kind="ExternalInput")
    attened = nc.dram_tensor("attened", [b_core, KEY_DIM], F32, kind="ExternalOutput")
    alpha_sm = nc.dram_tensor("alpha_sm", [b_core, N_KEYS], F32, kind="ExternalOutput")

    NC4 = N_KEYS // 128   # 4 n-chunks
    KC2 = KEY_DIM // 128  # 2 k-chunks
    DC4 = QUERY_DIM // 128  # 4 d-chunks

    with tile.TileContext(nc) as tc:
        with (
            tc.tile_pool(name="singles", bufs=1) as singles,
            tc.tile_pool(name="ktp", bufs=9) as ktp,
            tc.tile_pool(name="knp", bufs=9) as knp,
            tc.tile_pool(name="work", bufs=2) as work,
            tc.tile_pool(name="ps_tq", bufs=1, space="PSUM") as ps_tq,
            tc.tile_pool(name="ps_at", bufs=2, space="PSUM") as ps_at,
            tc.tile_pool(name="ps_al", bufs=1, space="PSUM") as ps_al,
            tc.tile_pool(name="ps_st", bufs=1, space="PSUM") as ps_st,
            tc.tile_pool(name="ps_kt", bufs=2, space="PSUM") as ps_kt,
            tc.tile_pool(name="ps_ka", bufs=1, space="PSUM") as ps_ka,
        ):
            qT_s = singles.tile([128, DC4, b_core], F32)
            nc.gpsimd.dma_start(qT_s, qT.rearrange("(c p) b -> p c b", p=128))
            wT_s = singles.tile([128, DC4, KEY_DIM], F32)
            nc.gpsimd.dma_start(wT_s, wT.rearrange("(c p) k -> p c k", p=128))
            bias_s = singles.tile([128, KC2], F32)
            nc.gpsimd.dma_start(bias_s, biasv.rearrange("(c p) -> p c", p=128))
            maskT_s = singles.tile([128, NC4, b_core], F32)
            nc.gpsimd.dma_start(maskT_s, maskT.rearrange("(c p) b -> p c b", p=128))
            idf_s = singles.tile([128, 128], F32)
            nc.gpsimd.dma_start(idf_s, identf[:, :])
            idh_s = singles.tile([128, 128], F16)
            nc.gpsimd.dma_start(idh_s, identh[:, :])

            for g in range(b_core // group):
                g0 = g * group

                # ---- linear: tqT[k, b] = W.T chunks.T @ qT chunks (fp32) ----
                tqT_ps = ps_tq.tile([128, KC2, group], F32, tag="tq")
                for kh in range(KC2):
                    for dc in range(DC4):
                        nc.tensor.matmul(
                            tqT_ps[:, kh, :],
                            lhsT=wT_s[:, dc, kh * 128:(kh + 1) * 128],
                            rhs=qT_s[:, dc, g0:g0 + group],
                            start=(kh == 0 and dc == 0),
                            stop=(kh == KC2 - 1 and dc == DC4 - 1),
                        )
                tqT_h = work.tile([128, KC2, group], F16, tag="tqh")
                for kh in range(KC2):
                    nc.vector.tensor_scalar_add(
                        tqT_h[:, kh, :], tqT_ps[:, kh, :], bias_s[:, kh:kh + 1]
                    )

                # ---- mm1: alphaT[n, b] += keysT_chunk.T @ tq col ----
                alphaT_ps = ps_at.tile([128, NC4, group], F32, tag="alphaT")
                for t in range(group // db):
                    kt_t = ktp.tile([128, db, KC2, N_KEYS], F16, tag="kt")
                    nc.sync.dma_start(
                        kt_t,
                        keyst[g0 // db + t].rearrange(
                            "p (b c n) -> p b c n", b=db, c=KC2
                        ),
                    )
                    for j in range(db):
                        jj = t * db + j
                        for nci in range(NC4):
                            for kc in range(KC2):
                                nc.tensor.matmul(
                                    alphaT_ps[:, nci, jj:jj + 1],
                                    lhsT=kt_t[:, j, kc, nci * 128:(nci + 1) * 128],
                                    rhs=tqT_h[:, kc, jj:jj + 1],
                                    start=(jj == 0 and nci == 0 and kc == 0),
                                    stop=(jj == group - 1 and nci == NC4 - 1
                                          and kc == KC2 - 1),
                                )

                # ---- mask add + evacuate PSUM -> SBUF ----
                alphaTm = work.tile([128, NC4, group], F32, tag="alphaTm")
                for c in range(NC4):
                    nc.vector.tensor_add(
                        alphaTm[:, c, :], alphaT_ps[:, c, :],
                        maskT_s[:, c, g0:g0 + group],
                    )

                # ---- transpose alphaT -> alpha [b, n] (PE, fp32) ----
                alpha_ps = ps_al.tile([group, N_KEYS], F32, tag="alpha")
                for c in range(NC4):
                    nc.tensor.matmul(
                        alpha_ps[:, c * 128:(c + 1) * 128],
                        lhsT=alphaTm[:, c, :], rhs=idf_s[:, :],
                        is_transpose=True,
                        start=(c == 0), stop=(c == NC4 - 1),
                    )

                # ---- softmax over n (rows = batches) ----
                negmax = work.tile([group, 1], F32, tag="negmax")
                nc.vector.reduce_max(
                    negmax, alpha_ps, axis=mybir.AxisListType.X, negate=True
                )
                expt = work.tile([group, N_KEYS], F32, tag="expt")
                sume = work.tile([group, 1], F32, tag="sume")
                nc.scalar.activation(
                    expt, alpha_ps, mybir.ActivationFunctionType.Exp,
                    bias=negmax, accum_out=sume,
                )
                rsum = work.tile([group, 1], F32, tag="rsum")
                nc.vector.reciprocal(rsum, sume)
                asm_f = work.tile([group, N_KEYS], F32, tag="asm_f")
                nc.vector.tensor_scalar_mul(asm_f, expt, rsum)
                asm_h = work.tile([group, N_KEYS], F16, tag="asm_h")
                nc.vector.tensor_scalar_mul(asm_h, expt, rsum)
                nc.scalar.dma_start(alpha_sm[g0:g0 + group, :], asm_f)

                # ---- transpose alpha_sm -> asmT [n, b] (PE, fp16) ----
                asmT_ps = ps_st.tile([128, NC4, group], F16, tag="asmT")
                for c in range(NC4):
                    nc.tensor.matmul(
                        asmT_ps[:, c, :],
                        lhsT=asm_h[:, c * 128:(c + 1) * 128],
                        rhs=idh_s[:group, :group],
                        is_transpose=True,
                        start=(c == 0), stop=(c == NC4 - 1),
                    )
                asmT_s = work.tile([128, NC4, group], F16, tag="asmT_s")
                for c in range(NC4):
                    nc.scalar.copy(asmT_s[:, c, :], asmT_ps[:, c, :])

                # ---- mm2: attT[k, b] += keysn_chunk.T @ asmT col ----
                attT_ps = ps_kt.tile([128, KC2, group], F32, tag="attT")
                for t in range(group // db):
                    kn_t = knp.tile([128, db, NC4, KEY_DIM], F16, tag="kn")
                    nc.sync.dma_start(
                        kn_t,
                        keysn[g0 // db + t].rearrange(
                            "p (b c k) -> p b c k", b=db, c=NC4
                        ),
                    )
                    for j in range(db):
                        jj = t * db + j
                        for kh in range(KC2):
                            for c in range(NC4):
                                nc.tensor.matmul(
                                    attT_ps[:, kh, jj:jj + 1],
                                    lhsT=kn_t[:, j, c, kh * 128:(kh + 1) * 128],
                                    rhs=asmT_s[:, c, jj:jj + 1],
                                    start=(jj == 0 and kh == 0 and c == 0),
                                    stop=(jj == group - 1 and kh == KC2 - 1
                                          and c == NC4 - 1),
                                )

                # ---- attened: evac, transpose to [b, k], store ----
                attT_s = work.tile([128, KC2, group], F32, tag="attT_s")
                for kh in range(KC2):
                    nc.scalar.copy(attT_s[:, kh, :], attT_ps[:, kh, :])
                att_ps = ps_ka.tile([group, KEY_DIM], F32, tag="att")
                for kh in range(KC2):
                    nc.tensor.matmul(
                        att_ps[:, kh * 128:(kh + 1) * 128],
                        lhsT=attT_s[:, kh, :], rhs=idf_s[:, :],
                        is_transpose=True,
                        start=(kh == 0), stop=(kh == KC2 - 1),
                    )
                att_s = work.tile([group, KEY_DIM], F32, tag="att_s")
                nc.scalar.copy(att_s, att_ps)
                nc.scalar.dma_start(attened[g0:g0 + group, :], att_s)

    nc.compile()
    return nc


_NC_CACHE = {}


def _get_nc():
    if "nc" not in _NC_CACHE:
        _NC_CACHE["nc"] = build()
    return _NC_CACHE["nc"]


def _install_trace_shims():
    """The agent image lacks antenv.axon_hooks; provide it so trace=True works,
    and stub the S3 artifact upload."""
    if "antenv.axon_hooks" not in sys.modules:
        import trn_agent_boot.trn_boot as tb

        mod = types.ModuleType("antenv.axon_hooks")
        hook = tb._ntff_profile_via_ctypes("/opt/axon/libaxon_pjrt.so")
        mod.get_axon_ntff_profile_hook = lambda: hook
        mod.set_axon_ntff_profile_hook = lambda h: None
        sys.modules["antenv.axon_hooks"] = mod
    bass_utils.upload_artifacts = lambda tmpdir: "local://skipped"


def pack_keysn(kn, db=4):
    """[B, 512, 256] f16 -> [B/db, 128, db*4*256] matching the SBUF tile
    layout [p, j, c, k] with n = c*128 + p."""
    B = kn.shape[0]
    v = kn.reshape(B // db, db, N_KEYS // 128, 128, KEY_DIM)
    v = v.transpose(0, 3, 1, 2, 4)
    return np.ascontiguousarray(v).reshape(B // db, 128, -1)


def pack_keyst(kt, db=4):
    """[B, 256, 512] f16 -> [B/db, 128, db*2*512] matching the SBUF tile
    layout [p, j, kc, n] with k = kc*128 + p."""
    B = kt.shape[0]
    v = kt.reshape(B // db, db, KEY_DIM // 128, 128, N_KEYS)
    v = v.transpose(0, 3, 1, 2, 4)
    return np.ascontiguousarray(v).reshape(B // db, 128, -1)


def make_in_maps(queries, keys, null_mask, W, b, n_cores=N_CORES, db=4):
    queries = np.asarray(queries, dtype=np.float32)
    keys = np.asarray(keys, dtype=np.float32)
    null_mask = np.asarray(null_mask)
    W = np.asarray(W, dtype=np.float32)
    b = np.asarray(b, dtype=np.float32)

    kn = keys.astype(np.float16)
    kt = np.ascontiguousarray(keys.transpose(0, 2, 1)).astype(np.float16)
    qT = np.ascontiguousarray(queries.T)
    wT = np.ascontiguousarray(W.T)
    maskT = np.ascontiguousarray(null_mask.T).astype(np.float32) * MASK_NEG
    identf = np.eye(128, dtype=np.float32)
    identh = np.eye(128, dtype=np.float16)

    nb = queries.shape[0]
    per = nb // n_cores
    in_maps = []
    for i in range(n_cores):
        sl = slice(i * per, (i + 1) * per)
        in_maps.append({
            "keysn": pack_keysn(kn[sl], db),
            "keyst": pack_keyst(kt[sl], db),
            "qT": np.ascontiguousarray(qT[:, sl]),
            "wT": wT,
            "biasv": b,
            "maskT": np.ascontiguousarray(maskT[:, sl]),
            "identf": identf,
            "identh": identh,
        })
    return in_maps


def kernel(queries, keys, trans_keys=None, null_mask=None, W=None, b=None):
    global LAST_EXEC_NS, LAST_RESULTS
    in_maps = make_in_maps(queries, keys, null_mask, W, b)
    nc = _get_nc()

    trace = os.environ.get("KERNEL_TRACE", "0") == "1"
    if trace:
        _install_trace_shims()
    res = bass_utils.run_bass_kernel_spmd(
        nc, in_maps, core_ids=list(range(N_CORES)), trace=trace
    )
    LAST_EXEC_NS = res.exec_time_ns
    LAST_RESULTS = res
    attened = np.concatenate([r["attened"] for r in res.results], axis=0)
    alpha_sm = np.concatenate([r["alpha_sm"] for r in res.results], axis=0)
    return attened.astype(np.float32), alpha_sm.astype(np.float32)
